# revision 1
# baseline (speedup 1.0000x reference)
"""Trainium2 Bass kernel for nn_Classifier_custom_12936441496172.

Reference math (per batch b, with av = column-l2-normalized img_b [Cf, R]):
    A      = softmax_r( (vv @ W1) @ av )          # [I, R] attention over R
    F_p    = A @ av.T                             # [I, Cf]
    out[b] = rowsum( (vv @ W2) * F_p )            # [I]

Identity used: out[b, i] = sum_r A[i, r] * ((vv @ W2) @ av)[i, r], so F_p is
never materialized. Q = vv@W1 and P = vv@W2 come from one stacked weight
matrix QPT (host-prepped; parameter-only work).

v6: TRANSPOSED-OUTPUT layout. The mains compute S.T: for each 128-row
r-chunk, stationary = x-block [128f x 128r], moving = qpt [128f, 624i],
giving PSUM tiles [128r, i]. With r on partitions:
- the softmax scale rn[r] = 1/||x[:, r]|| is PER-PARTITION, so it fuses for
  free into ACT Exp (scale=[128,1] AP) and into the P-side dot's
  scalar_tensor_tensor scalar operand. No rn broadcast, no drain
  multiplies, no ACT table flips (v1's Ln/Exp rn cost 2x 1.3us table loads
  per pair on the strict-FIFO ACT queue).
- n2[r] comes free from ACT Square with accum_out (free-axis = f sum) over
  a host-shipped transposed copy of img ([128r, 1024f] tiles), replacing
  v1's 8 squares + pair-adds + ones-matmul chain (~15 big elementwise ops
  per group). rsqrt is a DVE Newton iteration on [128, 4] columns
  (constant init y0=1/32; n2 ~ chi2(1024) is concentrated, 3 iters).
- softmax denominator sum_r E and output dot sum_r E*rn*pv are partition
  sums: accumulated per batch by one/two DVE adds (2 r-chunks per batch),
  then one ones-matmul each into a column-tiled PSUM tile, evacuated once
  per group.
DVE load drops to ~50%; the PE stream (64 main MMs + 4 small reduction MMs
per group) is the only bottleneck. Warmup: ~18 cold matmuls cover the
first x DMA (~8us); mains of group 0 start as soon as each k-chunk lands.
"""

import numpy as np

_PROGRAM = None

# Problem geometry (hardcoded per contract; kernel.py must be self-contained)
N_CORES = 8
NB = 16          # batches per core
R = 256          # H * W
CF = 1024        # feature channels
KC = CF // 128   # 8 contraction chunks
I = 312          # attributes
G = NB // 2      # groups of 2 batches
NR = 2 * R       # r-dim per group (2 batches)
RC = NR // 128   # 4 r-chunks per group
IA = 512         # A-tile columns (Q 0:312 | P 0:200)
IB = 2 * I - IA  # B-tile columns 112 (P 200:312)
PQ = IA - I      # 200: P columns in the A tile
W_WARM = 15


def _build_program():
    import concourse.tile as tile
    from concourse import bacc, mybir

    F32 = mybir.dt.float32
    BF16 = mybir.dt.bfloat16
    MULT = mybir.AluOpType.mult
    ADD = mybir.AluOpType.add
    EXP = mybir.ActivationFunctionType.Exp
    SQUARE = mybir.ActivationFunctionType.Square

    nc = bacc.Bacc(
        "TRN2",
        target_bir_lowering=False,
        debug=False,
        enable_asserts=False,
        num_devices=N_CORES,
    )
    img = nc.dram_tensor("img", [G, KC, 128, NR], BF16, kind="ExternalInput").ap()
    imgt = nc.dram_tensor("imgt", [G, RC, 128, CF], BF16, kind="ExternalInput").ap()
    qpt = nc.dram_tensor("qpt", [CF, 2 * I], BF16, kind="ExternalInput").ap()
    out = nc.dram_tensor("out", [NB, I], F32, kind="ExternalOutput").ap()

    with tile.TileContext(nc) as tc, tc.tile_pool(name="sb", bufs=2) as sb, tc.tile_pool(
        name="ps", bufs=6, space="PSUM"
    ) as ps:
        ones_b = nc.const_aps.tensor(1.0, (128, 1), BF16)

        def load_xn(g):
            xs = []
            for k in range(KC):
                x = sb.tile([128, NR], BF16, tag=f"x{k}", bufs=4, name=f"x{k}g{g}")
                nc.sync.dma_start(x[:, :], img[g, k])
                xs.append(x)
            return xs

        def load_xt(g):
            xt = []
            for rc in range(RC):
                t = sb.tile([128, CF], BF16, tag=f"xt{rc}", bufs=4, name=f"xt{rc}g{g}")
                nc.sync.dma_start(t[:, :], imgt[g, rc])
                xt.append(t)
            return xt

        def load_x(g):
            return load_xn(g), load_xt(g)

        # DMA order: x(0) alone first (it gates the first mains), qpt next,
        # then x(1), then the transposed copies (needed only by the rn
        # chains, ~3us after each group's mains begin).
        xs0 = load_xn(0)
        qpt_sb = sb.tile([128, KC * 2 * I], BF16, tag="qpt", bufs=1, name="qpt_sb")
        for k in range(KC):
            nc.sync.dma_start(
                qpt_sb[:, k * 2 * I : (k + 1) * 2 * I], qpt[k * 128 : (k + 1) * 128, :]
            )
        xt0 = load_xt(0)
        x_d = {0: (xs0, xt0), 1: load_x(1)}

        # Prime the single ACT table set (exp_and_others holds Exp+Square).
        prime = sb.tile([1, 16], F32, tag="prime", bufs=2, name="prime")
        nc.vector.memset(prime[:], 1.0)
        prime2 = sb.tile([1, 16], F32, tag="prime", bufs=2, name="prime2")
        nc.scalar.activation(prime2[:], prime[:], EXP)

        # Per-(group,batch) denominator / numerator rows, gathered at the end.
        dnsb = [
            sb.tile([97, I], F32, tag=f"dns{g}", bufs=1, name=f"dnsb{g}")
            for g in range(G)
        ]

        def n2_chain(g, xt):
            # n2 columns via ACT Square free-axis accumulation, then Newton
            # rsqrt on [128, RC] (parallel across partitions).
            n2c = sb.tile([128, RC], F32, tag="n2c", bufs=2, name=f"n2cg{g}")
            junk = sb.tile([128, CF], BF16, tag="junk", bufs=2, name=f"junkg{g}")
            for rc in range(RC):
                nc.scalar.activation(
                    junk[:], xt[rc][:], SQUARE, accum_out=n2c[:, rc : rc + 1]
                )
            def col(nm):
                return sb.tile([128, RC], F32, tag="nw", bufs=8, name=f"{nm}g{g}")

            Y0 = 0.03125
            u1 = col("u1")
            nc.vector.tensor_scalar(u1[:], n2c[:], -0.5 * Y0 * Y0, 1.5, MULT, ADD)
            y = col("y1")
            nc.vector.tensor_scalar_mul(y[:], u1[:], Y0)
            for it in range(2):
                t = col(f"t{it}")
                nc.vector.tensor_mul(t[:], y[:], y[:])
                t2 = col(f"t2{it}")
                nc.vector.tensor_mul(t2[:], t[:], n2c[:])
                u = col(f"u{it}")
                nc.vector.tensor_scalar(u[:], t2[:], -0.5, 1.5, MULT, ADD)
                yn = col(f"y{2 + it}")
                nc.vector.tensor_mul(yn[:], y[:], u[:])
                y = yn
            return y  # rn columns [128, RC] f32

        def main_rc(g, xs, rc):
            # A: [128r, 512] = x-block.T @ qpt[:, 0:512]; B: [128r, 112].
            a = ps.tile([128, IA], F32, tag="A", bufs=4, name=f"Ag{g}r{rc}")
            b = ps.tile([128, IB], F32, tag="B", bufs=3, name=f"Bg{g}r{rc}")
            for k in range(KC):
                blk = xs[k][:, rc * 128 : (rc + 1) * 128]
                nc.tensor.matmul(
                    a[:],
                    blk,
                    qpt_sb[:, k * 2 * I : k * 2 * I + IA],
                    start=(k == 0),
                    stop=(k == KC - 1),
                )
                nc.tensor.matmul(
                    b[:],
                    blk,
                    qpt_sb[:, k * 2 * I + IA : (k + 1) * 2 * I],
                    start=(k == 0),
                    stop=(k == KC - 1),
                )
            return a, b

        def drain_rc(g, rc, a, b, rnc):
            rcol = rnc[:, rc : rc + 1]
            E = sb.tile([128, I], BF16, tag="E", bufs=3, name=f"Eg{g}r{rc}")
            nc.scalar.activation(E[:], a[:, :I], EXP, scale=rcol)
            p1 = sb.tile([128, PQ], F32, tag="p1", bufs=3, name=f"p1g{g}r{rc}")
            nc.vector.scalar_tensor_tensor(
                out=p1[:], in0=E[:, :PQ], scalar=rcol, in1=a[:, I:IA],
                op0=MULT, op1=MULT,
            )
            p2 = sb.tile([128, IB], F32, tag="p2", bufs=3, name=f"p2g{g}r{rc}")
            nc.vector.scalar_tensor_tensor(
                out=p2[:], in0=E[:, PQ:I], scalar=rcol, in1=b[:, :],
                op0=MULT, op1=MULT,
            )
            return E, p1, p2

        def accum_batch(g, bi, d0, d1):
            # d0/d1: (E, p1, p2) of the two r-chunks of batch bi.
            den = sb.tile([128, I], BF16, tag="den", bufs=2, name=f"deng{g}b{bi}")
            nc.vector.tensor_add(den[:], d0[0][:], d1[0][:])
            num = sb.tile([128, I], BF16, tag="num", bufs=2, name=f"numg{g}b{bi}")
            nc.vector.tensor_add(num[:, :PQ], d0[1][:], d1[1][:])
            nc.vector.tensor_add(num[:, PQ:], d0[2][:], d1[2][:])
            return den, num

        def denum_mms(g, dn01):
            # Partition sums: 4 ones-matmuls into one column-tiled PSUM tile
            # (positions 0/32 = den b0/b1, 64/96 = num b0/b1), then one DVE
            # copy to the holding area and the row-gathers for the final
            # division (all overlapped with later groups).
            dn = ps.tile([97, I], F32, tag="dn", bufs=1, name=f"dng{g}")
            for bi in range(2):
                den, num = dn01[bi]
                nc.tensor.matmul(
                    dn[32 * bi : 32 * bi + 1, :], ones_b, den[:],
                    start=True, stop=True, tile_position=(0, 32 * bi),
                )
                nc.tensor.matmul(
                    dn[64 + 32 * bi : 64 + 32 * bi + 1, :], ones_b, num[:],
                    start=True, stop=True, tile_position=(0, 64 + 32 * bi),
                )
            nc.vector.tensor_scalar_mul(dnsb[g][:], dn[:], 1.0)
            for bi in range(2):
                nc.sync.dma_start(
                    dent[2 * g + bi : 2 * g + bi + 1, :],
                    dnsb[g][32 * bi : 32 * bi + 1, :],
                )
                nc.sync.dma_start(
                    numt[2 * g + bi : 2 * g + bi + 1, :],
                    dnsb[g][64 + 32 * bi : 64 + 32 * bi + 1, :],
                )


        # PE warm-up: cold matmuls covering the first x DMA (~8us) so the
        # HAM clock gate is released when the real stream begins.
        wsrc = sb.tile([128, IA], BF16, tag="warm", bufs=1, name="warmsrc")
        nc.vector.memset(wsrc[:], 0.0)
        wps = ps.tile([1, IA], F32, tag="dn", bufs=1, name="warmps")
        for i in range(W_WARM):
            nc.tensor.matmul(
                wps[:], ones_b, wsrc[:], start=(i == 0), stop=(i == W_WARM - 1)
            )

        dent = sb.tile([NB, I], F32, tag="dent", bufs=1, name="dent")
        numt = sb.tile([NB, I], F32, tag="numt", bufs=1, name="numt")
        rec = sb.tile([NB, I], F32, tag="rec", bufs=1, name="rec")

        rnc_d = {0: n2_chain(0, x_d[0][1])}
        pending = None  # previous group's (g, [dn_b0, dn_b1])

        for g in range(G):
            if g + 2 < G:
                x_d[g + 2] = load_x(g + 2)
            xs, _ = x_d.pop(g)
            drains = []
            for rc in range(RC):
                a, b = main_rc(g, xs, rc)
                if rc == 1 and pending is not None:
                    # previous group's reduction MMs slot in here, after its
                    # accum adds have certainly finished.
                    denum_mms(*pending)
                    pending = None
                drains.append(drain_rc(g, rc, a, b, rnc_d[g]))
                if rc == 1:
                    # next group's rn chain: its ACT ops run behind this
                    # group's first two Exp drains, well before needed.
                    if g + 1 < G:
                        rnc_d[g + 1] = n2_chain(g + 1, x_d[g + 1][1])
                    dn_b0 = accum_batch(g, 0, drains[0], drains[1])
            dn_b1 = accum_batch(g, 1, drains[2], drains[3])
            rnc_d.pop(g)
            pending = (g, [dn_b0, dn_b1])
        denum_mms(*pending)

        # Final division + store.
        nc.vector.reciprocal(rec[:], dent[:])
        fin = sb.tile([NB, I], F32, tag="fin", bufs=1, name="fin")
        nc.vector.tensor_mul(fin[:], numt[:], rec[:])
        nc.sync.dma_start(out[:, :], fin[:])

    nc.compile()
    return nc


def _prepare(inputs):
    img = np.asarray(inputs["img"], np.float32)
    V = np.asarray(inputs["V"], np.float32)
    W1 = np.asarray(inputs["W1"], np.float32)
    W2 = np.asarray(inputs["W2"], np.float32)
    B, Cf, H, W = img.shape
    assert (B, Cf, H * W) == (N_CORES * NB, CF, R), img.shape

    import ml_dtypes

    vv = V.astype(np.float64)
    vv /= np.maximum(np.sqrt((vv * vv).sum(1, keepdims=True)), 1e-12)
    Q = vv @ W1.astype(np.float64)  # [I, CF]
    P = vv @ W2.astype(np.float64)
    stacked = np.concatenate([Q, P], axis=0)  # [624, CF]
    qpt = np.ascontiguousarray(stacked.T.astype(ml_dtypes.bfloat16))  # [CF, 624]

    # Normal x: [G, KC, 128, 2R] (k-chunk rows x batch-pair columns) and a
    # transposed copy [G, RC, 128r, CF] for the ACT-accumulated norms.
    imgb = img.reshape(B, Cf, H * W).astype(ml_dtypes.bfloat16)
    imgb = imgb.reshape(N_CORES, G, 2, KC, 128, R).transpose(0, 1, 3, 4, 2, 5)
    imgb = np.ascontiguousarray(imgb.reshape(N_CORES, G, KC, 128, 2 * R))
    # imgt[c, g, rc, p, f] = x_cg[f, rc*128+p]
    imgt = imgb.reshape(N_CORES, G, KC, 128, 2 * R).transpose(0, 1, 4, 2, 3)
    imgt = np.ascontiguousarray(imgt.reshape(N_CORES, G, RC, 128, CF))
    in_maps = [
        {"img": imgb[c], "imgt": imgt[c], "qpt": qpt} for c in range(N_CORES)
    ]
    return in_maps


def run(inputs, **spmd_kwargs):
    """Run the kernel; returns (full_output [B, I], BassKernelResults)."""
    global _PROGRAM
    if _PROGRAM is None:
        _PROGRAM = _build_program()
    from concourse.bass_utils import run_bass_kernel_spmd

    in_maps = _prepare(inputs)
    res = run_bass_kernel_spmd(
        _PROGRAM, in_maps, core_ids=list(range(N_CORES)), **spmd_kwargs
    )
    out = np.concatenate(
        [np.asarray(res.results[c]["out"]) for c in range(N_CORES)], axis=0
    )
    return np.ascontiguousarray(out, np.float32), res


def kernel(**inputs) -> np.ndarray:
    return run(inputs)[0]

